# revision 10
# baseline (speedup 1.0000x reference)
"""Depthwise Conv3D (3x3x3, VALID, stride 1) on 8 Trainium2 NeuronCores.

Strategy: per-channel Toeplitz matmul over the H axis on TensorE.
  out[b,do,ho,wo,f] = sum_{kd,kh,kw} x[b,do+kd,ho+kh,wo+kw,f] * w[kd,kh,kw,f]
For fixed (f,kd,kw) the sum over kh is a banded [H_in=112, HO=110] Toeplitz
matrix applied along H, so one TensorE matmul (contraction over h_in on the
partition dim) handles all 3 kh taps; the 9 (kd,kw) combinations accumulate
in PSUM. Toeplitz matrices are built on the host from the tiny weight tensor.

Everything runs in bf16 (1 cycle/row on the PE, same as fp32r, but half the
DMA bytes; products accumulate in fp32 PSUM so rel err stays ~1e-3). The
moving operand is a 2D [7 d, 56 w] strided view of the slab so no junk
columns are streamed (392 rows/matmul). Outputs are staged and DMA'd in
bf16 and upcast to fp32 on the host.

Sharding: (batch, channel-half) -> 8 shards. Channel sharding halves the
per-core Toeplitz traffic (it scales with resident channels); the Toeplitz
for a core's 32 channels is loaded once per kernel pass and stays resident
in SBUF (~63 KB/partition). Within a core, D is processed in two
7-output-plane chunks (9 input planes each) and W in two 58-column halves
(56 output columns each) so a PSUM bank holds a full [110, 7, 56] f32 tile.
"""

import sys

sys.path.insert(0, "/opt/trn_rl_repo")

from contextlib import ExitStack

import numpy as np

B, D, H, W, F = 4, 16, 112, 112, 64
DO, HO, WO = 14, 110, 110
N_CORES = 8
FC = 32  # channels per core
DO_C = 7  # output d-planes per d-chunk
DIN_C = 9  # input d-planes per d-chunk
WIN = 58  # input w columns per half
WEV = 56  # output wo columns per half
W_SPLITS = [0, 54]  # w start of each half (both input and output)
FQ = 4  # channels per DMA batch
NQ = FC // FQ  # 8 DMA batches per core
TPF = 9 * HO + 18  # toeplitz elems per (h, f): 9 taps x 110 + 18 zero pad so a
# [112, 128]-column stationary view exists for every tap (FWL needs 128 cols)

DBG_SKIP_OUT = False  # ablation: drop output DMAs
DBG_SKIP_MM = False  # ablation: drop matmuls + evacs
DBG_SKIP_IN = False  # ablation: drop input DMAs (slab+toep); results garbage

_cached = None


def _build(loop_n: int = 1, skip: tuple = ()):
    # skip: subset of {"out", "mm1", "inx2", "toepx2"} — timing ablations
    from concourse import bacc, mybir, tile

    nc = bacc.Bacc("TRN2", target_bir_lowering=False, debug=False, num_devices=N_CORES)
    f32 = mybir.dt.float32
    bf16 = mybir.dt.bfloat16

    x_ap = nc.dram_tensor(
        "xp", [2, 2, H, FC, DIN_C, WIN], bf16, kind="ExternalInput"
    ).ap()
    t_ap = nc.dram_tensor(
        "toep", [NQ, H, FQ, TPF], bf16, kind="ExternalInput"
    ).ap()
    b_ap = nc.dram_tensor("biasbc", [128, FC], f32, kind="ExternalInput").ap()
    o_ap = nc.dram_tensor("out", [DO, HO, WO, FC], bf16, kind="ExternalOutput").ap()

    with tile.TileContext(nc) as tc, ExitStack() as ctx:
        slab_pool = ctx.enter_context(tc.tile_pool(name="slab", bufs=5))
        toep_pool = ctx.enter_context(tc.tile_pool(name="toep", bufs=1))
        stage_pool = ctx.enter_context(tc.tile_pool(name="stage", bufs=2))
        psum_pool = ctx.enter_context(tc.tile_pool(name="psum", bufs=8, space="PSUM"))
        const_pool = ctx.enter_context(tc.tile_pool(name="const", bufs=1))

        bias_t = const_pool.tile([128, FC], f32, name="bias_t")
        nc.sync.dma_start(out=bias_t[:], in_=b_ap[:])

        loop_ctx = tc.For_i(0, loop_n) if loop_n > 1 else None
        if loop_ctx is not None:
            ctx.enter_context(loop_ctx)

        toep_q = [None] * NQ
        for ih, w0 in enumerate(W_SPLITS):
            for dc in range(2):
                stage = stage_pool.tile(
                    [HO, DO_C, WEV, FC], bf16, name="stage", tag="stage"
                )
                for q in range(NQ):
                    if ih == 0 and dc == 0:
                        # Toeplitz loads go on the Pool HWDGE ring: they fire
                        # once per pass and mustn't queue behind the slab
                        # stream (ACT ring) or the output drain (SP ring).
                        toep_q[q] = toep_pool.tile(
                            [H, FQ, TPF], bf16, name="toep_q", tag=f"tq{q}"
                        )
                        teng = nc.gpsimd if q % 2 == 0 else nc.sync
                        teng.dma_start(out=toep_q[q][:], in_=t_ap[q])
                        if "toepx2" in skip:
                            toep_x = toep_pool.tile(
                                [H, FQ, TPF], bf16, name="toep_x", tag="tx"
                            )
                            nc.gpsimd.dma_start(out=toep_x[:], in_=t_ap[q])
                    slab_q = slab_pool.tile(
                        [H, FQ, DIN_C, WIN], bf16, name="slab_q", tag="sq"
                    )
                    nc.scalar.dma_start(
                        out=slab_q[:], in_=x_ap[dc, ih, :, q * FQ : (q + 1) * FQ]
                    )
                    if "inx2" in skip:
                        slab_x = slab_pool.tile(
                            [H, FQ, DIN_C, WIN], bf16, name="slab_x", tag="sx"
                        )
                        nc.scalar.dma_start(
                            out=slab_x[:], in_=x_ap[dc, ih, :, q * FQ : (q + 1) * FQ]
                        )
                    for fi in range(FQ):
                        f = q * FQ + fi
                        psum_t = psum_pool.tile(
                            [128, DO_C, WEV], f32, name="psum_t", tag="ps"
                        )
                        ntap = 1 if "mm1" in skip else 9
                        for kd in range(3):
                            for kw in range(3):
                                tap = kd * 3 + kw
                                if tap >= ntap:
                                    continue
                                nc.tensor.matmul(
                                    psum_t[:],
                                    lhsT=toep_q[q][:, fi, tap * HO : tap * HO + 128],
                                    rhs=slab_q[:, fi, kd : kd + DO_C, kw : kw + WEV],
                                    start=(tap == 0),
                                    stop=(tap == ntap - 1),
                                )
                        # evacuate PSUM -> staging, add bias, cast to bf16
                        # (DVE is ~2x faster per evac than ACT -> 2:1 split)
                        if f % 3 != 2:
                            nc.vector.tensor_scalar_add(
                                stage[:, :, :, f],
                                psum_t[0:HO],
                                bias_t[0:HO, f : f + 1],
                            )
                        else:
                            nc.scalar.activation(
                                stage[:, :, :, f],
                                psum_t[0:HO],
                                mybir.ActivationFunctionType.Identity,
                                bias=bias_t[0:HO, f : f + 1],
                            )
                for do in range(DO_C):
                    if "out" in skip:
                        break
                    nc.sync.dma_start(
                        out=o_ap[dc * DO_C + do, :, w0 : w0 + WEV, :],
                        in_=stage[:, do],
                    )

    nc.compile()
    return nc


def _np_bf16():
    import ml_dtypes

    return ml_dtypes.bfloat16


def _toeplitz(w: np.ndarray) -> np.ndarray:
    """[3,3,3,1,F] -> per-half-F quad-batched toeplitz [2][NQ, H, FQ, TPF]."""
    t = np.zeros((F, H, TPF), np.float32)
    ho = np.arange(HO)
    for kd in range(3):
        for kh in range(3):
            for kw in range(3):
                t[:, ho + kh, (kd * 3 + kw) * HO + ho] = w[kd, kh, kw, 0, :][:, None]
    t = t.astype(_np_bf16())
    halves = []
    for fh in range(2):
        th = t[fh * FC : (fh + 1) * FC]  # [FC, H, TPF]
        th = np.ascontiguousarray(
            th.reshape(NQ, FQ, H, TPF).transpose(0, 2, 1, 3)
        )
        halves.append(th)
    return halves


def _pack_x(xs: np.ndarray) -> np.ndarray:
    """[D, H, W, FC] -> [2, 2, H, FC, DIN_C, WIN] (dchunk, whalf, h, f, d, w)."""
    xp = np.empty((2, 2, H, FC, DIN_C, WIN), _np_bf16())
    for dc in range(2):
        for ih, w0 in enumerate(W_SPLITS):
            chunk = xs[dc * DO_C : dc * DO_C + DIN_C, :, w0 : w0 + WIN, :]
            xp[dc, ih] = chunk.transpose(1, 3, 0, 2)
    return xp


def kernel(x: np.ndarray, w: np.ndarray, b: np.ndarray) -> np.ndarray:
    global _cached
    if _cached is None:
        _cached = _build()
    nc = _cached

    from concourse.bass_utils import run_bass_kernel_spmd

    x = np.asarray(x, np.float32)
    toep = _toeplitz(np.asarray(w, np.float32))
    b = np.asarray(b, np.float32)

    in_maps = []
    for core in range(N_CORES):
        bb, fh = divmod(core, 2)
        in_maps.append(
            {
                "xp": _pack_x(x[bb, :, :, :, fh * FC : (fh + 1) * FC]),
                "toep": toep[fh],
                "biasbc": np.tile(b[None, fh * FC : (fh + 1) * FC], (128, 1)),
            }
        )

    res = run_bass_kernel_spmd(nc, in_maps, list(range(N_CORES)))

    out = np.empty((B, DO, HO, WO, F), np.float32)
    for core in range(N_CORES):
        bb, fh = divmod(core, 2)
        out[bb, :, :, :, fh * FC : (fh + 1) * FC] = res.results[core]["out"].astype(
            np.float32
        )
    return out


# revision 11
# speedup vs baseline: 1.0285x; 1.0285x over previous
"""Depthwise Conv3D (3x3x3, VALID, stride 1) on 8 Trainium2 NeuronCores.

Strategy: per-channel Toeplitz matmul over the H axis on TensorE.
  out[b,do,ho,wo,f] = sum_{kd,kh,kw} x[b,do+kd,ho+kh,wo+kw,f] * w[kd,kh,kw,f]
For fixed (f,kd,kw) the sum over kh is a banded [H_in=112, HO=110] Toeplitz
matrix applied along H, so one TensorE matmul (contraction over h_in on the
partition dim) handles all 3 kh taps; the 9 (kd,kw) combinations accumulate
in PSUM. Toeplitz matrices are built on the host from the tiny weight tensor.

Everything runs in bf16 (1 cycle/row on the PE, same as fp32r, but half the
DMA bytes; products accumulate in fp32 PSUM so rel err stays ~1e-3). The
moving operand is a 2D [7 d, 56 w] strided view of the slab so no junk
columns are streamed (392 rows/matmul). Outputs are staged and DMA'd in
bf16 and upcast to fp32 on the host.

Sharding: (batch, channel-half) -> 8 shards. Channel sharding halves the
per-core Toeplitz traffic (it scales with resident channels); the Toeplitz
for a core's 32 channels is loaded once per kernel pass and stays resident
in SBUF (~63 KB/partition). Within a core, D is processed in two
7-output-plane chunks (9 input planes each) and W in two 58-column halves
(56 output columns each) so a PSUM bank holds a full [110, 7, 56] f32 tile.
"""

import sys

sys.path.insert(0, "/opt/trn_rl_repo")

from contextlib import ExitStack

import numpy as np

B, D, H, W, F = 4, 16, 112, 112, 64
DO, HO, WO = 14, 110, 110
N_CORES = 8
FC = 32  # channels per core
DO_C = 7  # output d-planes per d-chunk
DIN_C = 9  # input d-planes per d-chunk
WIN = 58  # input w columns per half
WEV = 56  # output wo columns per half
W_SPLITS = [0, 54]  # w start of each half (both input and output)
FQ = 4  # channels per DMA batch
NQ = FC // FQ  # 8 DMA batches per core
TPF = 9 * HO + 18  # toeplitz elems per (h, f): 9 taps x 110 + 18 zero pad so a
# [112, 128]-column stationary view exists for every tap (FWL needs 128 cols)

DBG_SKIP_OUT = False  # ablation: drop output DMAs
DBG_SKIP_MM = False  # ablation: drop matmuls + evacs
DBG_SKIP_IN = False  # ablation: drop input DMAs (slab+toep); results garbage

_cached = None


def _build(loop_n: int = 1, skip: tuple = ()):
    # skip: subset of {"out", "mm1", "inx2", "toepx2"} — timing ablations
    from concourse import bacc, mybir, tile

    nc = bacc.Bacc("TRN2", target_bir_lowering=False, debug=False, num_devices=N_CORES)
    f32 = mybir.dt.float32
    bf16 = mybir.dt.bfloat16

    x_ap = nc.dram_tensor(
        "xp", [2, 2, H, FC, DIN_C, WIN], bf16, kind="ExternalInput"
    ).ap()
    t_ap = nc.dram_tensor(
        "toep", [NQ, H, FQ, TPF], bf16, kind="ExternalInput"
    ).ap()
    b_ap = nc.dram_tensor("biasbc", [128, FC], f32, kind="ExternalInput").ap()
    o_ap = nc.dram_tensor("out", [DO, HO, WO, FC], bf16, kind="ExternalOutput").ap()

    with tile.TileContext(nc) as tc, ExitStack() as ctx:
        slab_pool = ctx.enter_context(tc.tile_pool(name="slab", bufs=5))
        toep_pool = ctx.enter_context(tc.tile_pool(name="toep", bufs=1))
        stage_pool = ctx.enter_context(tc.tile_pool(name="stage", bufs=2))
        psum_pool = ctx.enter_context(tc.tile_pool(name="psum", bufs=8, space="PSUM"))
        const_pool = ctx.enter_context(tc.tile_pool(name="const", bufs=1))

        bias_t = const_pool.tile([128, FC], f32, name="bias_t")
        nc.sync.dma_start(out=bias_t[:], in_=b_ap[:])

        loop_ctx = tc.For_i(0, loop_n) if loop_n > 1 else None
        if loop_ctx is not None:
            ctx.enter_context(loop_ctx)

        toep_q = [None] * NQ
        for ih, w0 in enumerate(W_SPLITS):
            for dc in range(2):
                stage = stage_pool.tile(
                    [HO, DO_C, WEV, FC], bf16, name="stage", tag="stage"
                )
                for q in range(NQ):
                    if ih == 0 and dc == 0:
                        # Toeplitz loads go on the Pool HWDGE ring: they fire
                        # once per pass and mustn't queue behind the slab
                        # stream (ACT ring) or the output drain (SP ring).
                        toep_q[q] = toep_pool.tile(
                            [H, FQ, TPF], bf16, name="toep_q", tag=f"tq{q}"
                        )
                        nc.gpsimd.dma_start(out=toep_q[q][:], in_=t_ap[q])
                        if "toepx2" in skip:
                            toep_x = toep_pool.tile(
                                [H, FQ, TPF], bf16, name="toep_x", tag="tx"
                            )
                            nc.gpsimd.dma_start(out=toep_x[:], in_=t_ap[q])
                    slab_q = slab_pool.tile(
                        [H, FQ, DIN_C, WIN], bf16, name="slab_q", tag="sq"
                    )
                    nc.scalar.dma_start(
                        out=slab_q[:], in_=x_ap[dc, ih, :, q * FQ : (q + 1) * FQ]
                    )
                    if "inx2" in skip:
                        slab_x = slab_pool.tile(
                            [H, FQ, DIN_C, WIN], bf16, name="slab_x", tag="sx"
                        )
                        nc.scalar.dma_start(
                            out=slab_x[:], in_=x_ap[dc, ih, :, q * FQ : (q + 1) * FQ]
                        )
                    for fi in range(FQ):
                        f = q * FQ + fi
                        psum_t = psum_pool.tile(
                            [128, DO_C, WEV], f32, name="psum_t", tag="ps"
                        )
                        ntap = 1 if "mm1" in skip else 9
                        for kd in range(3):
                            for kw in range(3):
                                tap = kd * 3 + kw
                                if tap >= ntap:
                                    continue
                                nc.tensor.matmul(
                                    psum_t[:],
                                    lhsT=toep_q[q][:, fi, tap * HO : tap * HO + 128],
                                    rhs=slab_q[:, fi, kd : kd + DO_C, kw : kw + WEV],
                                    start=(tap == 0),
                                    stop=(tap == ntap - 1),
                                )
                        # evacuate PSUM -> staging, add bias, cast to bf16
                        # (DVE is ~2x faster per evac than ACT -> 2:1 split)
                        if f % 3 != 2:
                            nc.vector.tensor_scalar_add(
                                stage[:, :, :, f],
                                psum_t[0:HO],
                                bias_t[0:HO, f : f + 1],
                            )
                        else:
                            nc.scalar.activation(
                                stage[:, :, :, f],
                                psum_t[0:HO],
                                mybir.ActivationFunctionType.Identity,
                                bias=bias_t[0:HO, f : f + 1],
                            )
                for do in range(DO_C):
                    if "out" in skip:
                        break
                    nc.sync.dma_start(
                        out=o_ap[dc * DO_C + do, :, w0 : w0 + WEV, :],
                        in_=stage[:, do],
                    )

    nc.compile()
    return nc


def _np_bf16():
    import ml_dtypes

    return ml_dtypes.bfloat16


def _toeplitz(w: np.ndarray) -> np.ndarray:
    """[3,3,3,1,F] -> per-half-F quad-batched toeplitz [2][NQ, H, FQ, TPF]."""
    t = np.zeros((F, H, TPF), np.float32)
    ho = np.arange(HO)
    for kd in range(3):
        for kh in range(3):
            for kw in range(3):
                t[:, ho + kh, (kd * 3 + kw) * HO + ho] = w[kd, kh, kw, 0, :][:, None]
    t = t.astype(_np_bf16())
    halves = []
    for fh in range(2):
        th = t[fh * FC : (fh + 1) * FC]  # [FC, H, TPF]
        th = np.ascontiguousarray(
            th.reshape(NQ, FQ, H, TPF).transpose(0, 2, 1, 3)
        )
        halves.append(th)
    return halves


def _pack_x(xs: np.ndarray) -> np.ndarray:
    """[D, H, W, FC] -> [2, 2, H, FC, DIN_C, WIN] (dchunk, whalf, h, f, d, w)."""
    xp = np.empty((2, 2, H, FC, DIN_C, WIN), _np_bf16())
    for dc in range(2):
        for ih, w0 in enumerate(W_SPLITS):
            chunk = xs[dc * DO_C : dc * DO_C + DIN_C, :, w0 : w0 + WIN, :]
            xp[dc, ih] = chunk.transpose(1, 3, 0, 2)
    return xp


def kernel(x: np.ndarray, w: np.ndarray, b: np.ndarray) -> np.ndarray:
    global _cached
    if _cached is None:
        _cached = _build()
    nc = _cached

    from concourse.bass_utils import run_bass_kernel_spmd

    x = np.asarray(x, np.float32)
    toep = _toeplitz(np.asarray(w, np.float32))
    b = np.asarray(b, np.float32)

    in_maps = []
    for core in range(N_CORES):
        bb, fh = divmod(core, 2)
        in_maps.append(
            {
                "xp": _pack_x(x[bb, :, :, :, fh * FC : (fh + 1) * FC]),
                "toep": toep[fh],
                "biasbc": np.tile(b[None, fh * FC : (fh + 1) * FC], (128, 1)),
            }
        )

    res = run_bass_kernel_spmd(nc, in_maps, list(range(N_CORES)))

    out = np.empty((B, DO, HO, WO, F), np.float32)
    for core in range(N_CORES):
        bb, fh = divmod(core, 2)
        out[bb, :, :, :, fh * FC : (fh + 1) * FC] = res.results[core]["out"].astype(
            np.float32
        )
    return out
